# revision 31
# baseline (speedup 1.0000x reference)
"""MoE routed expert matmul on 8 Trainium2 NeuronCores.

Problem: out[n] = input[n] @ w[inds[n]] + b[inds[n]]
  input [262144, 32] f32, inds [262144] i32 (1024 experts), w [1024, 32, 32], b [1024, 1, 32]

Strategy (K-stacked expert quads; host does routing/layout only — all FLOPs
on device):
  * Host sorts the 1024 experts by global token count (ascending) and chunks
    them into 32 quad-groups of 32 experts with near-equal counts.  Chunk q
    supplies one expert to each (core, band) pair: expert chunks[q][4k + r]
    goes to core k, quad q, band r (r in 0..3).  Every core runs the same
    program over its own 32 quads; quad q's column width Q[q] = max token
    count in the chunk (global max, so the SPMD shapes match), rounded up to
    8.  Count-matched chunks keep padding to a few percent.
  * Activation layout xt [128, TOTW] fp16: token t of (quad q, band r) sits
    at column X[q] + t, rows 32r..32r+32 (its 32 features).  Each column
    carries up to 4 tokens (one per band) — full 128-row density.
  * Weights upload as block-diagonal K=64 stacks (wq, 0.5 MB): for each quad
    and half h, a [64, 64] tile holds experts (q, 2h) and (q, 2h+1) on the
    diagonal.  Two [K=64, M=64, N=Q] matmuls per quad (tile_position (0,0) /
    (64,64)) then compute all 4 bands' tokens — each activation column
    streams through the PE twice instead of 4x (vs per-expert 32x32 tiles),
    and the off-diagonal zeros kill the cross-expert terms.
  * The PSUM result + per-quad bias column goes to an fp16 output tile
    (Scalar/Vector alternating), stored to DRAM in per-4-quad groups on
    alternating DMA rings (GpSimd SWDGE / Scalar HWDGE).  fp16 I/O halves
    DMA traffic vs f32; per-core HBM bytes ~4.8 MB -> ~13.3 us at 360 B/ns.
  * Host scatters the sorted outputs back to original token order.

Layouts (core k, quad q, band r = 2h + s, expert e = chunks[q][4k + r]):
  xt [128, TOTW]  xt[32r+i, X[q] + t]        = x[token t of e, feat i]  (fp16)
  wq [128, 2048]  wq[64h+32s+i, 64q+32s+o]   = w[e, i, o], 0 off-diag   (fp16)
  bp [128, 32]    bp[32r+o, q]               = b[e, 0, o]               (f32)
  ot [128, TOTW]  ot[32r+o, X[q] + t]        = out[token t of e, feat o](fp16)
"""

import numpy as np

import concourse.bass as bass
import concourse.mybir as mybir
import concourse.tile as tile
from concourse import bacc
from concourse.bass_utils import run_bass_kernel_spmd

N_TOK = 262144
E = 1024
F = 32
O = 32
NCORES = 8
NQUAD = 32  # quads per core; 4 experts each = 128 experts/core
GQ = 4  # quads per load/store group
NG = NQUAD // GQ
F32 = mybir.dt.float32
MM_DT = mybir.dt.float16
OT_DT = mybir.dt.float16

N_WARM = 8  # PE ramp warm-up matmuls
WARM_N = 160  # free-dim length of each warm-up matmul

_programs: dict[tuple, "bacc.Bacc"] = {}


class _CapacityOverflow(Exception):
    """A single expert got >512 tokens (~16 sigma out for uniform routing at
    256 tokens/expert).  Handled by a host fallback so kernel() still
    returns a correct result."""


def _plan(counts):
    """Chunk experts into count-matched quads; per-quad widths and offsets."""
    order_e = np.argsort(counts, kind="stable")  # ascending counts
    # chunk q holds 32 count-matched experts; descending so the pipeline
    # tail (last-stored groups) drains on the smallest transfers
    chunks = order_e.reshape(NQUAD, 32)[::-1]
    Q = np.maximum(16, ((counts[chunks[:, -1]] + 7) // 8) * 8)  # [NQUAD]
    # quads 2-3 of each group share one batched DVE bias op, which needs a
    # uniform column stride — pad that pair to its max (sorted chunks make
    # this ~0.5%); quads 0-1 get per-quad ACT ops and stay exact
    Q4 = Q.reshape(NG, GQ).copy()
    Q4[:, 2:4] = Q4[:, 2:4].max(axis=1, keepdims=True)
    Q = Q4.reshape(-1)
    if Q.max() > 512:
        raise _CapacityOverflow(int(counts.max()))
    X = np.zeros(NQUAD + 1, dtype=np.int64)
    np.cumsum(Q, out=X[1:])
    TOTW = int(X[-1])
    j = np.arange(32)
    e_quad = np.empty(E, dtype=np.int64)
    e_core = np.empty(E, dtype=np.int64)
    e_band = np.empty(E, dtype=np.int64)
    e_quad[chunks] = np.arange(NQUAD)[:, None]
    e_core[chunks] = (j // 4)[None, :]
    e_band[chunks] = (j % 4)[None, :]
    return Q.astype(np.int64), X, TOTW, e_quad, e_core, e_band


def _build(Q, X, TOTW) -> "bacc.Bacc":
    nc = bacc.Bacc("TRN2", target_bir_lowering=False, debug=False, num_devices=NCORES)
    xt = nc.declare_dram_parameter("xt", [128, TOTW], MM_DT, isOutput=False)
    wq = nc.declare_dram_parameter("wq", [128, NQUAD * 64], MM_DT, isOutput=False)
    bp = nc.declare_dram_parameter("bp", [128, NQUAD], F32, isOutput=False)
    ot = nc.declare_dram_parameter("ot", [128, TOTW], OT_DT, isOutput=True)

    with tile.TileContext(nc) as tc:
        with (
            tc.tile_pool(name="const", bufs=1) as c_pool,
            tc.tile_pool(name="xt", bufs=NG) as xt_pool,
            tc.tile_pool(name="out", bufs=NG) as out_pool,
            tc.tile_pool(name="psm", bufs=4, space="PSUM") as psm_pool,
        ):
            wq_t = c_pool.tile([128, NQUAD * 64], MM_DT)
            bp_t = c_pool.tile([128, NQUAD], F32)
            warm_t = c_pool.tile([128, WARM_N], MM_DT)

            # loads: gpsimd (SWDGE, 25ns SEQ issue) carries wq in chunks —
            # groups 0-1 first so compute starts early — plus the bias;
            # sync (SP HWDGE) carries all xt
            wq2g = 2 * GQ * 64
            nc.gpsimd.dma_start(out=wq_t[:, :wq2g], in_=wq[:, :wq2g])
            nc.gpsimd.dma_start(out=bp_t[:], in_=bp[:])
            nc.gpsimd.dma_start(out=wq_t[:, wq2g:], in_=wq[:, wq2g:])

            # PE ramp warm-up on a memset scratch tile (PSUM never read);
            # the dummy activation pulls ACT's 1.3us LoadActFuncSet into the
            # load phase instead of stalling the first real bias op
            nc.vector.memset(warm_t[:], 0.0)
            nc.scalar.activation(
                warm_t[0:1, 0:1],
                warm_t[0:1, 0:1],
                mybir.ActivationFunctionType.Identity,
                bias=warm_t[0:1, 1:2],
                scale=1.0,
            )
            warm_ps = psm_pool.tile(
                [128, WARM_N], F32, space="PSUM", name="warm_ps", tag="psm"
            )
            for _ in range(N_WARM):
                nc.tensor.matmul(
                    out=warm_ps[0:32, :],
                    lhsT=warm_t[0:32, 0:32],
                    rhs=warm_t[0:32, :],
                    start=True,
                    stop=True,
                    tile_position=(0, 0),
                )

            xt_tiles = {}
            o_tiles = {}

            def load_group(g, cuts=()):
                a, bnd = int(X[GQ * g]), int(X[GQ * (g + 1)])
                t = xt_pool.tile([128, bnd - a], MM_DT, name="xt_t", tag="xt_t")
                for c0, c1 in zip((a, *cuts), (*cuts, bnd)):
                    nc.sync.dma_start(
                        out=t[:, c0 - a : c1 - a], in_=xt[:, c0:c1]
                    )
                xt_tiles[g] = t

            # all xt tiles are resident; loads issue upfront and run
            # back-to-back so stores queue behind them and the compute tail
            # hides inside the store backlog
            load_group(0, cuts=(int(X[1]),))
            for g in range(1, NG):
                load_group(g)

            for g in range(NG):
                a, bnd = int(X[GQ * g]), int(X[GQ * (g + 1)])
                o_t = out_pool.tile([128, bnd - a], OT_DT, name="o_t", tag="o_t")
                # pair-level PSUM tiles (2 banks each, 4 in flight) keep the
                # bias latency out of the PSUM-recycle critical loop
                for pi in range(2):
                    psm = psm_pool.tile(
                        [128, 2 * 512], F32, space="PSUM", name="psm", tag="psm"
                    )
                    for si in range(2):
                        q = GQ * g + 2 * pi + si
                        Qq = int(Q[q])
                        off = int(X[q] - a)
                        for h in range(2):
                            nc.tensor.matmul(
                                out=psm[
                                    64 * h : 64 * h + 64, 512 * si : 512 * si + Qq
                                ],
                                lhsT=wq_t[64 * h : 64 * h + 64, 64 * q : 64 * q + 64],
                                rhs=xt_tiles[g][
                                    64 * h : 64 * h + 64, off : off + Qq
                                ],
                                start=True,
                                stop=True,
                                tile_position=(64 * h, 64 * h),
                            )
                    # bias + fp16 down-convert: ACT takes the first pair as
                    # two exact-width activation ops (it is the serial/slower
                    # engine, so give it the earlier-ready pair); DVE takes
                    # the second pair in one tensor_tensor (broadcast bias,
                    # pair-uniform width) — parallel engines, different banks
                    if pi == 0:
                        for si in range(2):
                            q = GQ * g + si
                            Qq = int(Q[q])
                            off = int(X[q] - a)
                            nc.scalar.activation(
                                o_t[:, off : off + Qq],
                                psm[:, 512 * si : 512 * si + Qq],
                                mybir.ActivationFunctionType.Identity,
                                bias=bp_t[:, q : q + 1],
                                scale=1.0,
                            )
                    else:
                        Qp = int(Q[GQ * g + 2])
                        off = int(X[GQ * g + 2] - a)
                        psm_view = psm[:, :].rearrange("p (c t) -> p c t", c=2)[
                            :, :, :Qp
                        ]
                        bias_view = bp_t[
                            :, GQ * g + 2 : GQ * g + 4, None
                        ].to_broadcast([128, 2, Qp])
                        out_view = o_t[:, off : off + 2 * Qp].rearrange(
                            "p (c t) -> p c t", c=2
                        )
                        nc.vector.tensor_tensor(
                            out=out_view,
                            in0=psm_view,
                            in1=bias_view,
                            op=mybir.AluOpType.add,
                        )
                # stores all ride the SP ring: they queue behind the loads in
                # the same SEQ/HWDGE FIFO, so loads keep strict DMA priority
                # (no store steals a slot mid-load-phase) and the compute
                # drain starts as early as possible
                nc.sync.dma_start(out=ot[:, a:bnd], in_=o_t[:])

    nc.compile()
    return nc


def _pack(x, inds, w, b):
    """Host-side routing: sort tokens by expert, build per-core device arrays."""
    counts = np.bincount(inds, minlength=E)
    Q, X, TOTW, e_quad, e_core, e_band = _plan(counts)

    order = np.argsort(inds, kind="stable")
    sorted_inds = inds[order]
    starts = np.zeros(E, dtype=np.int64)
    np.cumsum(counts[:-1], out=starts[1:])
    slot = np.arange(N_TOK, dtype=np.int64) - starts[sorted_inds]

    k_tok = e_core[sorted_inds]
    r_tok = e_band[sorted_inds]
    col_tok = X[e_quad[sorted_inds]] + slot

    mdt = mybir.dt.np(MM_DT)
    xt_all = np.zeros((NCORES, 4, F, TOTW), dtype=mdt)
    xt_all[k_tok, r_tok, :, col_tok] = x[order].astype(mdt)
    xt = xt_all.reshape(NCORES, 128, TOTW)

    # wq[k, h, s, i, q, s', o] = w[e, i, o] on the s == s' diagonal
    e_half = e_band // 2
    e_sub = e_band % 2
    wqn = np.zeros((NCORES, 2, 2, F, NQUAD, 2, O), dtype=mdt)
    wqn[e_core, e_half, e_sub, :, e_quad, e_sub, :] = w.astype(mdt)
    wqk = wqn.reshape(NCORES, 128, NQUAD * 64)

    bpn = np.zeros((NCORES, 4, O, NQUAD), dtype=np.float32)
    bpn[e_core, e_band, :, e_quad] = b[:, 0, :]
    bpk = bpn.reshape(NCORES, 128, NQUAD)

    plan = (Q, X, TOTW)
    return plan, order, (k_tok, r_tok, col_tok), xt, wqk, bpk


def _unpack(results, tok_addr, order):
    k_tok, r_tok, col_tok = tok_addr
    ot = np.stack([results[k]["ot"] for k in range(NCORES)])  # [k, 128, TOTW]
    ot4 = ot.reshape(NCORES, 4, O, -1)  # [k, r, o, col]
    out = np.empty((N_TOK, O), dtype=np.float32)
    out[order] = ot4[k_tok, r_tok, :, col_tok]
    return out


def _prepare(x, inds, w, b):
    """Pack inputs and return (nc, in_maps, tok_addr, order)."""
    plan, order, tok_addr, xt, wqk, bpk = _pack(x, inds, w, b)
    Q, X, TOTW = plan
    key = (MM_DT, OT_DT, Q.tobytes())
    nc = _programs.get(key)
    if nc is None:
        nc = _build(Q, X, TOTW)
        _programs[key] = nc
    in_maps = [{"xt": xt[k], "wq": wqk[k], "bp": bpk[k]} for k in range(NCORES)]
    return nc, in_maps, tok_addr, order


def kernel(input, inds, w, b):
    x = np.ascontiguousarray(np.asarray(input, dtype=np.float32))
    inds = np.asarray(inds, dtype=np.int32)
    w = np.ascontiguousarray(np.asarray(w, dtype=np.float32))
    b = np.ascontiguousarray(np.asarray(b, dtype=np.float32))
    assert x.shape == (N_TOK, F) and inds.shape == (N_TOK,)
    assert w.shape == (E, F, O) and b.shape == (E, 1, O)

    try:
        nc, in_maps, tok_addr, order = _prepare(x, inds, w, b)
    except _CapacityOverflow:
        return (np.einsum("ni,nio->no", x, w[inds]) + b[inds, 0]).astype(np.float32)

    res = run_bass_kernel_spmd(nc, in_maps, list(range(NCORES)))
    return _unpack(res.results, tok_addr, order)


def last_program():
    """The most recently compiled Bass program (for profiling in test.py)."""
    return next(iter(_programs.values())) if _programs else None


# revision 37
# speedup vs baseline: 1.0145x; 1.0145x over previous
"""MoE routed expert matmul on 8 Trainium2 NeuronCores.

Problem: out[n] = input[n] @ w[inds[n]] + b[inds[n]]
  input [262144, 32] f32, inds [262144] i32 (1024 experts), w [1024, 32, 32], b [1024, 1, 32]

Strategy (K-stacked expert quads; host does routing/layout only — all FLOPs
on device):
  * Host sorts the 1024 experts by global token count (ascending) and chunks
    them into 32 quad-groups of 32 experts with near-equal counts.  Chunk q
    supplies one expert to each (core, band) pair: expert chunks[q][4k + r]
    goes to core k, quad q, band r (r in 0..3).  Every core runs the same
    program over its own 32 quads; quad q's column width Q[q] = max token
    count in the chunk (global max, so the SPMD shapes match), rounded up to
    8.  Count-matched chunks keep padding to a few percent.
  * Activation layout xt [128, TOTW] fp16: token t of (quad q, band r) sits
    at column X[q] + t, rows 32r..32r+32 (its 32 features).  Each column
    carries up to 4 tokens (one per band) — full 128-row density.
  * Weights upload as block-diagonal K=64 stacks (wq, 0.5 MB): for each quad
    and half h, a [64, 64] tile holds experts (q, 2h) and (q, 2h+1) on the
    diagonal.  Two [K=64, M=64, N=Q] matmuls per quad (tile_position (0,0) /
    (64,64)) then compute all 4 bands' tokens — each activation column
    streams through the PE twice instead of 4x (vs per-expert 32x32 tiles),
    and the off-diagonal zeros kill the cross-expert terms.
  * The PSUM result + per-quad bias column goes to an fp16 output tile
    (Scalar/Vector alternating), stored to DRAM in per-4-quad groups on
    alternating DMA rings (GpSimd SWDGE / Scalar HWDGE).  fp16 I/O halves
    DMA traffic vs f32; per-core HBM bytes ~4.8 MB -> ~13.3 us at 360 B/ns.
  * Host scatters the sorted outputs back to original token order.

Layouts (core k, quad q, band r = 2h + s, expert e = chunks[q][4k + r]):
  xt [128, TOTW]  xt[32r+i, X[q] + t]        = x[token t of e, feat i]  (fp16)
  wq [128, 2048]  wq[64h+32s+i, 64q+32s+o]   = w[e, i, o], 0 off-diag   (fp16)
  bp [128, 32]    bp[32r+o, q]               = b[e, 0, o]               (f32)
  ot [128, TOTW]  ot[32r+o, X[q] + t]        = out[token t of e, feat o](fp16)
"""

import numpy as np

import concourse.bass as bass
import concourse.mybir as mybir
import concourse.tile as tile
from concourse import bacc
from concourse.bass_utils import run_bass_kernel_spmd

N_TOK = 262144
E = 1024
F = 32
O = 32
NCORES = 8
NQUAD = 32  # quads per core; 4 experts each = 128 experts/core
GQ = 4  # quads per load/store group
NG = NQUAD // GQ
F32 = mybir.dt.float32
MM_DT = mybir.dt.float16
OT_DT = mybir.dt.float16

N_WARM = 6  # PE ramp warm-up matmuls
WARM_N = 160  # free-dim length of each warm-up matmul
STORE_GPSIMD_N = 2  # leading store groups on the Pool SWDGE ring (rest: SP)
SPLIT_LAST = True  # split the last group's load at its final quad

_programs: dict[tuple, "bacc.Bacc"] = {}


class _CapacityOverflow(Exception):
    """A single expert got >512 tokens (~16 sigma out for uniform routing at
    256 tokens/expert).  Handled by a host fallback so kernel() still
    returns a correct result."""


def _plan(counts):
    """Chunk experts into count-matched quads; per-quad widths and offsets."""
    order_e = np.argsort(counts, kind="stable")  # ascending counts
    # chunk q holds 32 count-matched experts; descending so the pipeline
    # tail (last-stored groups) drains on the smallest transfers
    chunks = order_e.reshape(NQUAD, 32)[::-1]
    Q = np.maximum(16, ((counts[chunks[:, -1]] + 7) // 8) * 8)  # [NQUAD]
    # quads 2-3 of each group share one batched DVE bias op, which needs a
    # uniform column stride — pad that pair to its max (sorted chunks make
    # this ~0.5%); quads 0-1 get per-quad ACT ops and stay exact
    Q4 = Q.reshape(NG, GQ).copy()
    Q4[:, 2:4] = Q4[:, 2:4].max(axis=1, keepdims=True)
    Q = Q4.reshape(-1)
    if Q.max() > 512:
        raise _CapacityOverflow(int(counts.max()))
    X = np.zeros(NQUAD + 1, dtype=np.int64)
    np.cumsum(Q, out=X[1:])
    TOTW = int(X[-1])
    j = np.arange(32)
    e_quad = np.empty(E, dtype=np.int64)
    e_core = np.empty(E, dtype=np.int64)
    e_band = np.empty(E, dtype=np.int64)
    e_quad[chunks] = np.arange(NQUAD)[:, None]
    e_core[chunks] = (j // 4)[None, :]
    e_band[chunks] = (j % 4)[None, :]
    return Q.astype(np.int64), X, TOTW, e_quad, e_core, e_band


def _build(Q, X, TOTW) -> "bacc.Bacc":
    nc = bacc.Bacc("TRN2", target_bir_lowering=False, debug=False, num_devices=NCORES)
    xt = nc.declare_dram_parameter("xt", [128, TOTW], MM_DT, isOutput=False)
    wq = nc.declare_dram_parameter("wq", [128, NQUAD * 64], MM_DT, isOutput=False)
    bp = nc.declare_dram_parameter("bp", [128, NQUAD], F32, isOutput=False)
    ot = nc.declare_dram_parameter("ot", [128, TOTW], OT_DT, isOutput=True)

    with tile.TileContext(nc) as tc:
        with (
            tc.tile_pool(name="const", bufs=1) as c_pool,
            tc.tile_pool(name="xt", bufs=NG) as xt_pool,
            tc.tile_pool(name="out", bufs=NG) as out_pool,
            tc.tile_pool(name="psm", bufs=4, space="PSUM") as psm_pool,
        ):
            wq_t = c_pool.tile([128, NQUAD * 64], MM_DT)
            bp_t = c_pool.tile([128, NQUAD], F32)
            warm_t = c_pool.tile([128, WARM_N], MM_DT)

            # loads: gpsimd (SWDGE, 25ns SEQ issue) carries wq in chunks —
            # groups 0-1 first so compute starts early — plus the bias;
            # sync (SP HWDGE) carries all xt
            wq2g = 2 * GQ * 64
            nc.gpsimd.dma_start(out=wq_t[:, :wq2g], in_=wq[:, :wq2g])
            nc.gpsimd.dma_start(out=bp_t[:], in_=bp[:])
            nc.gpsimd.dma_start(out=wq_t[:, wq2g:], in_=wq[:, wq2g:])

            # PE ramp warm-up on a memset scratch tile (PSUM never read);
            # the dummy activation pulls ACT's 1.3us LoadActFuncSet into the
            # load phase instead of stalling the first real bias op
            nc.vector.memset(warm_t[:], 0.0)
            nc.scalar.activation(
                warm_t[0:1, 0:1],
                warm_t[0:1, 0:1],
                mybir.ActivationFunctionType.Identity,
                bias=warm_t[0:1, 1:2],
                scale=1.0,
            )
            warm_ps = psm_pool.tile(
                [128, WARM_N], F32, space="PSUM", name="warm_ps", tag="psm"
            )
            for _ in range(N_WARM):
                nc.tensor.matmul(
                    out=warm_ps[0:32, :],
                    lhsT=warm_t[0:32, 0:32],
                    rhs=warm_t[0:32, :],
                    start=True,
                    stop=True,
                    tile_position=(0, 0),
                )

            xt_tiles = {}
            o_tiles = {}

            def load_group(g, cuts=()):
                a, bnd = int(X[GQ * g]), int(X[GQ * (g + 1)])
                t = xt_pool.tile([128, bnd - a], MM_DT, name="xt_t", tag="xt_t")
                for c0, c1 in zip((a, *cuts), (*cuts, bnd)):
                    nc.sync.dma_start(
                        out=t[:, c0 - a : c1 - a], in_=xt[:, c0:c1]
                    )
                xt_tiles[g] = t

            # all xt tiles are resident; loads issue upfront and run
            # back-to-back so stores queue behind them and the compute tail
            # hides inside the store backlog
            # group 0 split at the first quad so compute starts early
            load_group(0, cuts=(int(X[1]),))
            for g in range(1, NG - 1):
                load_group(g)
            load_group(NG - 1, cuts=(int(X[NQUAD - 1]),) if SPLIT_LAST else ())

            for g in range(NG):
                a, bnd = int(X[GQ * g]), int(X[GQ * (g + 1)])
                o_t = out_pool.tile([128, bnd - a], OT_DT, name="o_t", tag="o_t")
                # pair-level PSUM tiles (2 banks each, 4 in flight) keep the
                # bias latency out of the PSUM-recycle critical loop
                for pi in range(2):
                    psm = psm_pool.tile(
                        [128, 2 * 512], F32, space="PSUM", name="psm", tag="psm"
                    )
                    for si in range(2):
                        q = GQ * g + 2 * pi + si
                        Qq = int(Q[q])
                        off = int(X[q] - a)
                        for h in range(2):
                            nc.tensor.matmul(
                                out=psm[
                                    64 * h : 64 * h + 64, 512 * si : 512 * si + Qq
                                ],
                                lhsT=wq_t[64 * h : 64 * h + 64, 64 * q : 64 * q + 64],
                                rhs=xt_tiles[g][
                                    64 * h : 64 * h + 64, off : off + Qq
                                ],
                                start=True,
                                stop=True,
                                tile_position=(64 * h, 64 * h),
                            )
                    # bias + fp16 down-convert: ACT takes the first pair as
                    # two exact-width activation ops (it is the serial/slower
                    # engine, so give it the earlier-ready pair); DVE takes
                    # the second pair in one tensor_tensor (broadcast bias,
                    # pair-uniform width) — parallel engines, different banks
                    if pi == 0:
                        for si in range(2):
                            q = GQ * g + si
                            Qq = int(Q[q])
                            off = int(X[q] - a)
                            nc.scalar.activation(
                                o_t[:, off : off + Qq],
                                psm[:, 512 * si : 512 * si + Qq],
                                mybir.ActivationFunctionType.Identity,
                                bias=bp_t[:, q : q + 1],
                                scale=1.0,
                            )
                    else:
                        Qp = int(Q[GQ * g + 2])
                        off = int(X[GQ * g + 2] - a)
                        psm_view = psm[:, :].rearrange("p (c t) -> p c t", c=2)[
                            :, :, :Qp
                        ]
                        bias_view = bp_t[
                            :, GQ * g + 2 : GQ * g + 4, None
                        ].to_broadcast([128, 2, Qp])
                        out_view = o_t[:, off : off + 2 * Qp].rearrange(
                            "p (c t) -> p c t", c=2
                        )
                        nc.vector.tensor_tensor(
                            out=out_view,
                            in0=psm_view,
                            in1=bias_view,
                            op=mybir.AluOpType.add,
                        )
                # stores: leading groups on the Pool SWDGE ring (25ns SEQ
                # issue, never blocks compute); the rest on the SP ring where
                # they queue behind the loads (loads keep DMA priority) and
                # get the lower-latency HWDGE issue path for the tail
                ring = nc.gpsimd if g < STORE_GPSIMD_N else nc.sync
                ring.dma_start(out=ot[:, a:bnd], in_=o_t[:])

    nc.compile()
    return nc


def _pack(x, inds, w, b):
    """Host-side routing: sort tokens by expert, build per-core device arrays."""
    counts = np.bincount(inds, minlength=E)
    Q, X, TOTW, e_quad, e_core, e_band = _plan(counts)

    order = np.argsort(inds, kind="stable")
    sorted_inds = inds[order]
    starts = np.zeros(E, dtype=np.int64)
    np.cumsum(counts[:-1], out=starts[1:])
    slot = np.arange(N_TOK, dtype=np.int64) - starts[sorted_inds]

    k_tok = e_core[sorted_inds]
    r_tok = e_band[sorted_inds]
    col_tok = X[e_quad[sorted_inds]] + slot

    mdt = mybir.dt.np(MM_DT)
    xt_all = np.zeros((NCORES, 4, F, TOTW), dtype=mdt)
    xt_all[k_tok, r_tok, :, col_tok] = x[order].astype(mdt)
    xt = xt_all.reshape(NCORES, 128, TOTW)

    # wq[k, h, s, i, q, s', o] = w[e, i, o] on the s == s' diagonal
    e_half = e_band // 2
    e_sub = e_band % 2
    wqn = np.zeros((NCORES, 2, 2, F, NQUAD, 2, O), dtype=mdt)
    wqn[e_core, e_half, e_sub, :, e_quad, e_sub, :] = w.astype(mdt)
    wqk = wqn.reshape(NCORES, 128, NQUAD * 64)

    bpn = np.zeros((NCORES, 4, O, NQUAD), dtype=np.float32)
    bpn[e_core, e_band, :, e_quad] = b[:, 0, :]
    bpk = bpn.reshape(NCORES, 128, NQUAD)

    plan = (Q, X, TOTW)
    return plan, order, (k_tok, r_tok, col_tok), xt, wqk, bpk


def _unpack(results, tok_addr, order):
    k_tok, r_tok, col_tok = tok_addr
    ot = np.stack([results[k]["ot"] for k in range(NCORES)])  # [k, 128, TOTW]
    ot4 = ot.reshape(NCORES, 4, O, -1)  # [k, r, o, col]
    out = np.empty((N_TOK, O), dtype=np.float32)
    out[order] = ot4[k_tok, r_tok, :, col_tok]
    return out


def _prepare(x, inds, w, b):
    """Pack inputs and return (nc, in_maps, tok_addr, order)."""
    plan, order, tok_addr, xt, wqk, bpk = _pack(x, inds, w, b)
    Q, X, TOTW = plan
    key = (MM_DT, OT_DT, N_WARM, WARM_N, STORE_GPSIMD_N, SPLIT_LAST, Q.tobytes())
    nc = _programs.get(key)
    if nc is None:
        nc = _build(Q, X, TOTW)
        _programs[key] = nc
    in_maps = [{"xt": xt[k], "wq": wqk[k], "bp": bpk[k]} for k in range(NCORES)]
    return nc, in_maps, tok_addr, order


def kernel(input, inds, w, b):
    x = np.ascontiguousarray(np.asarray(input, dtype=np.float32))
    inds = np.asarray(inds, dtype=np.int32)
    w = np.ascontiguousarray(np.asarray(w, dtype=np.float32))
    b = np.ascontiguousarray(np.asarray(b, dtype=np.float32))
    assert x.shape == (N_TOK, F) and inds.shape == (N_TOK,)
    assert w.shape == (E, F, O) and b.shape == (E, 1, O)

    try:
        nc, in_maps, tok_addr, order = _prepare(x, inds, w, b)
    except _CapacityOverflow:
        return (np.einsum("ni,nio->no", x, w[inds]) + b[inds, 0]).astype(np.float32)

    res = run_bass_kernel_spmd(nc, in_maps, list(range(NCORES)))
    return _unpack(res.results, tok_addr, order)


def last_program():
    """The most recently compiled Bass program (for profiling in test.py)."""
    return next(iter(_programs.values())) if _programs else None


# revision 45
# speedup vs baseline: 1.0232x; 1.0086x over previous
"""MoE routed expert matmul on 8 Trainium2 NeuronCores.

Problem: out[n] = input[n] @ w[inds[n]] + b[inds[n]]
  input [262144, 32] f32, inds [262144] i32 (1024 experts), w [1024, 32, 32], b [1024, 1, 32]

Strategy (K-stacked expert quads; host does routing/layout only — all FLOPs
on device):
  * Host sorts the 1024 experts by global token count (ascending) and chunks
    them into 32 quad-groups of 32 experts with near-equal counts.  Chunk q
    supplies one expert to each (core, band) pair: expert chunks[q][4k + r]
    goes to core k, quad q, band r (r in 0..3).  Every core runs the same
    program over its own 32 quads; quad q's column width Q[q] = max token
    count in the chunk (global max, so the SPMD shapes match), rounded up to
    8.  Count-matched chunks keep padding to a few percent.
  * Activation layout xt [128, TOTW] fp16: token t of (quad q, band r) sits
    at column X[q] + t, rows 32r..32r+32 (its 32 features).  Each column
    carries up to 4 tokens (one per band) — full 128-row density.
  * Weights upload as block-diagonal K=64 stacks (wq, 0.5 MB): for each quad
    and half h, a [64, 64] tile holds experts (q, 2h) and (q, 2h+1) on the
    diagonal.  Two [K=64, M=64, N=Q] matmuls per quad (tile_position (0,0) /
    (64,64)) then compute all 4 bands' tokens — each activation column
    streams through the PE twice instead of 4x (vs per-expert 32x32 tiles),
    and the off-diagonal zeros kill the cross-expert terms.
  * Matmuls accumulate into pair-level PSUM tiles (2 quads x 512-col banks,
    4 in flight) so the bias latency stays out of the PSUM-recycle loop.
    Bias + fp16 down-convert runs split across engines per 4-quad group:
    ScalarE handles quads 0-1 as two exact-width activation ops, VectorE
    quads 2-3 in one tensor_tensor with a broadcast bias view.
  * Schedule: all xt tiles are SBUF-resident; loads issue upfront on the SP
    ring and run back-to-back, stores queue behind them (2 early groups on
    the GpSimd SWDGE ring, the rest on SP), so the DMA engines never idle
    and the compute tail hides inside the store backlog.  Group processing
    order pulls two small groups early so the drain-phase bias backlog
    never paces the final stores.  fp16 I/O halves DMA traffic vs f32;
    per-core HBM bytes ~4.8 MB -> ~13.5 us at 360 B/ns, sim 17.1 us.
  * Host scatters the sorted outputs back to original token order.

Layouts (core k, quad q, band r = 2h + s, expert e = chunks[q][4k + r]):
  xt [128, TOTW]  xt[32r+i, X[q] + t]        = x[token t of e, feat i]  (fp16)
  wq [128, 2048]  wq[64h+32s+i, 64q+32s+o]   = w[e, i, o], 0 off-diag   (fp16)
  bp [128, 32]    bp[32r+o, q]               = b[e, 0, o]               (fp16)
  ot [128, TOTW]  ot[32r+o, X[q] + t]        = out[token t of e, feat o](fp16)
"""

import numpy as np

import concourse.bass as bass
import concourse.mybir as mybir
import concourse.tile as tile
from concourse import bacc
from concourse.bass_utils import run_bass_kernel_spmd

N_TOK = 262144
E = 1024
F = 32
O = 32
NCORES = 8
NQUAD = 32  # quads per core; 4 experts each = 128 experts/core
GQ = 4  # quads per load/store group
NG = NQUAD // GQ
F32 = mybir.dt.float32
MM_DT = mybir.dt.float16
OT_DT = mybir.dt.float16

N_WARM = 6  # PE ramp warm-up matmuls
WARM_N = 160  # free-dim length of each warm-up matmul
STORE_GPSIMD_N = 2  # leading store groups on the Pool SWDGE ring (rest: SP)
SPLIT_LAST = True  # split the last group's load at its final quad
# group processing order (indices into size-descending groups): two small
# groups early so the engine bias backlog never paces the store drain
GROUP_ORDER = (0, 6, 7, 1, 2, 3, 4, 5)

_programs: dict[tuple, "bacc.Bacc"] = {}


class _CapacityOverflow(Exception):
    """A single expert got >512 tokens (~16 sigma out for uniform routing at
    256 tokens/expert).  Handled by a host fallback so kernel() still
    returns a correct result."""


def _plan(counts):
    """Chunk experts into count-matched quads; per-quad widths and offsets."""
    order_e = np.argsort(counts, kind="stable")  # ascending counts
    # chunk q holds 32 count-matched experts; descending so the pipeline
    # tail (last-stored groups) drains on the smallest transfers
    chunks = order_e.reshape(NQUAD, 32)[::-1]
    # optional group-level processing permutation (load/compute/store order)
    chunks = chunks.reshape(NG, GQ, 32)[list(GROUP_ORDER)].reshape(NQUAD, 32)
    Q = np.maximum(16, ((counts[chunks[:, -1]] + 7) // 8) * 8)  # [NQUAD]
    # quads 2-3 of each group share one batched DVE bias op, which needs a
    # uniform column stride — pad that pair to its max (sorted chunks make
    # this ~0.5%); quads 0-1 get per-quad ACT ops and stay exact
    Q4 = Q.reshape(NG, GQ).copy()
    Q4[:, 2:4] = Q4[:, 2:4].max(axis=1, keepdims=True)
    Q = Q4.reshape(-1)
    if Q.max() > 512:
        raise _CapacityOverflow(int(counts.max()))
    X = np.zeros(NQUAD + 1, dtype=np.int64)
    np.cumsum(Q, out=X[1:])
    TOTW = int(X[-1])
    j = np.arange(32)
    e_quad = np.empty(E, dtype=np.int64)
    e_core = np.empty(E, dtype=np.int64)
    e_band = np.empty(E, dtype=np.int64)
    e_quad[chunks] = np.arange(NQUAD)[:, None]
    e_core[chunks] = (j // 4)[None, :]
    e_band[chunks] = (j % 4)[None, :]
    return Q.astype(np.int64), X, TOTW, e_quad, e_core, e_band


def _build(Q, X, TOTW) -> "bacc.Bacc":
    nc = bacc.Bacc("TRN2", target_bir_lowering=False, debug=False, num_devices=NCORES)
    xt = nc.declare_dram_parameter("xt", [128, TOTW], MM_DT, isOutput=False)
    wq = nc.declare_dram_parameter("wq", [128, NQUAD * 64], MM_DT, isOutput=False)
    bp = nc.declare_dram_parameter("bp", [128, NQUAD], MM_DT, isOutput=False)
    ot = nc.declare_dram_parameter("ot", [128, TOTW], OT_DT, isOutput=True)

    with tile.TileContext(nc) as tc:
        with (
            tc.tile_pool(name="const", bufs=1) as c_pool,
            tc.tile_pool(name="xt", bufs=NG) as xt_pool,
            tc.tile_pool(name="out", bufs=NG) as out_pool,
            tc.tile_pool(name="psm", bufs=4, space="PSUM") as psm_pool,
        ):
            wq_t = c_pool.tile([128, NQUAD * 64], MM_DT)
            bp_t = c_pool.tile([128, NQUAD], MM_DT)
            warm_t = c_pool.tile([128, WARM_N], MM_DT)

            # loads: gpsimd (SWDGE, 25ns SEQ issue) carries wq in chunks —
            # groups 0-1 first so compute starts early — plus the bias;
            # sync (SP HWDGE) carries all xt
            wq2g = 2 * GQ * 64
            nc.gpsimd.dma_start(out=wq_t[:, :wq2g], in_=wq[:, :wq2g])
            nc.gpsimd.dma_start(out=bp_t[:], in_=bp[:])
            nc.gpsimd.dma_start(out=wq_t[:, wq2g:], in_=wq[:, wq2g:])

            # PE ramp warm-up on a memset scratch tile (PSUM never read);
            # the dummy activation pulls ACT's 1.3us LoadActFuncSet into the
            # load phase instead of stalling the first real bias op
            nc.vector.memset(warm_t[:], 0.0)
            nc.scalar.activation(
                warm_t[0:1, 0:1],
                warm_t[0:1, 0:1],
                mybir.ActivationFunctionType.Identity,
                bias=warm_t[0:1, 1:2],
                scale=1.0,
            )
            warm_ps = psm_pool.tile(
                [128, WARM_N], F32, space="PSUM", name="warm_ps", tag="psm"
            )
            for _ in range(N_WARM):
                nc.tensor.matmul(
                    out=warm_ps[0:32, :],
                    lhsT=warm_t[0:32, 0:32],
                    rhs=warm_t[0:32, :],
                    start=True,
                    stop=True,
                    tile_position=(0, 0),
                )

            xt_tiles = {}
            o_tiles = {}

            def load_group(g, cuts=()):
                a, bnd = int(X[GQ * g]), int(X[GQ * (g + 1)])
                t = xt_pool.tile([128, bnd - a], MM_DT, name="xt_t", tag="xt_t")
                for c0, c1 in zip((a, *cuts), (*cuts, bnd)):
                    nc.sync.dma_start(
                        out=t[:, c0 - a : c1 - a], in_=xt[:, c0:c1]
                    )
                xt_tiles[g] = t

            # all xt tiles are resident; loads issue upfront and run
            # back-to-back so stores queue behind them and the compute tail
            # hides inside the store backlog
            # group 0 split at the first quad so compute starts early
            load_group(0, cuts=(int(X[1]),))
            for g in range(1, NG - 1):
                load_group(g)
            # keep the trailing piece >= 256 cols (512B rows) so it doesn't
            # pay the sub-512B descriptor latency penalty
            last_cut = min(int(X[NQUAD - 1]), int(X[NQUAD]) - 256)
            split_ok = SPLIT_LAST and last_cut > int(X[GQ * (NG - 1)])
            load_group(NG - 1, cuts=(last_cut,) if split_ok else ())

            for g in range(NG):
                a, bnd = int(X[GQ * g]), int(X[GQ * (g + 1)])
                o_t = out_pool.tile([128, bnd - a], OT_DT, name="o_t", tag="o_t")
                # pair-level PSUM tiles (2 banks each, 4 in flight) keep the
                # bias latency out of the PSUM-recycle critical loop
                for pi in range(2):
                    psm = psm_pool.tile(
                        [128, 2 * 512], F32, space="PSUM", name="psm", tag="psm"
                    )
                    for si in range(2):
                        q = GQ * g + 2 * pi + si
                        Qq = int(Q[q])
                        off = int(X[q] - a)
                        for h in range(2):
                            nc.tensor.matmul(
                                out=psm[
                                    64 * h : 64 * h + 64, 512 * si : 512 * si + Qq
                                ],
                                lhsT=wq_t[64 * h : 64 * h + 64, 64 * q : 64 * q + 64],
                                rhs=xt_tiles[g][
                                    64 * h : 64 * h + 64, off : off + Qq
                                ],
                                start=True,
                                stop=True,
                                tile_position=(64 * h, 64 * h),
                            )
                    # bias + fp16 down-convert: ACT takes the first pair as
                    # two exact-width activation ops (it is the serial/slower
                    # engine, so give it the earlier-ready pair); DVE takes
                    # the second pair in one tensor_tensor (broadcast bias,
                    # pair-uniform width) — parallel engines, different banks
                    if pi == 0:
                        for si in range(2):
                            q = GQ * g + si
                            Qq = int(Q[q])
                            off = int(X[q] - a)
                            nc.scalar.activation(
                                o_t[:, off : off + Qq],
                                psm[:, 512 * si : 512 * si + Qq],
                                mybir.ActivationFunctionType.Identity,
                                bias=bp_t[:, q : q + 1],
                                scale=1.0,
                            )
                    else:
                        Qp = int(Q[GQ * g + 2])
                        off = int(X[GQ * g + 2] - a)
                        psm_view = psm[:, :].rearrange("p (c t) -> p c t", c=2)[
                            :, :, :Qp
                        ]
                        bias_view = bp_t[
                            :, GQ * g + 2 : GQ * g + 4, None
                        ].to_broadcast([128, 2, Qp])
                        out_view = o_t[:, off : off + 2 * Qp].rearrange(
                            "p (c t) -> p c t", c=2
                        )
                        nc.vector.tensor_tensor(
                            out=out_view,
                            in0=psm_view,
                            in1=bias_view,
                            op=mybir.AluOpType.add,
                        )
                # stores: leading groups on the Pool SWDGE ring (25ns SEQ
                # issue, never blocks compute); the rest on the SP ring where
                # they queue behind the loads (loads keep DMA priority) and
                # get the lower-latency HWDGE issue path for the tail
                ring = nc.gpsimd if g < STORE_GPSIMD_N else nc.sync
                ring.dma_start(out=ot[:, a:bnd], in_=o_t[:])

    nc.compile()
    return nc


def _pack(x, inds, w, b):
    """Host-side routing: sort tokens by expert, build per-core device arrays."""
    counts = np.bincount(inds, minlength=E)
    Q, X, TOTW, e_quad, e_core, e_band = _plan(counts)

    order = np.argsort(inds, kind="stable")
    sorted_inds = inds[order]
    starts = np.zeros(E, dtype=np.int64)
    np.cumsum(counts[:-1], out=starts[1:])
    slot = np.arange(N_TOK, dtype=np.int64) - starts[sorted_inds]

    k_tok = e_core[sorted_inds]
    r_tok = e_band[sorted_inds]
    col_tok = X[e_quad[sorted_inds]] + slot

    mdt = mybir.dt.np(MM_DT)
    xt_all = np.zeros((NCORES, 4, F, TOTW), dtype=mdt)
    xt_all[k_tok, r_tok, :, col_tok] = x[order].astype(mdt)
    xt = xt_all.reshape(NCORES, 128, TOTW)

    # wq[k, h, s, i, q, s', o] = w[e, i, o] on the s == s' diagonal
    e_half = e_band // 2
    e_sub = e_band % 2
    wqn = np.zeros((NCORES, 2, 2, F, NQUAD, 2, O), dtype=mdt)
    wqn[e_core, e_half, e_sub, :, e_quad, e_sub, :] = w.astype(mdt)
    wqk = wqn.reshape(NCORES, 128, NQUAD * 64)

    bpn = np.zeros((NCORES, 4, O, NQUAD), dtype=mdt)
    bpn[e_core, e_band, :, e_quad] = b[:, 0, :].astype(mdt)
    bpk = bpn.reshape(NCORES, 128, NQUAD)

    plan = (Q, X, TOTW)
    return plan, order, (k_tok, r_tok, col_tok), xt, wqk, bpk


def _unpack(results, tok_addr, order):
    k_tok, r_tok, col_tok = tok_addr
    ot = np.stack([results[k]["ot"] for k in range(NCORES)])  # [k, 128, TOTW]
    ot4 = ot.reshape(NCORES, 4, O, -1)  # [k, r, o, col]
    out = np.empty((N_TOK, O), dtype=np.float32)
    out[order] = ot4[k_tok, r_tok, :, col_tok]
    return out


def _prepare(x, inds, w, b):
    """Pack inputs and return (nc, in_maps, tok_addr, order)."""
    plan, order, tok_addr, xt, wqk, bpk = _pack(x, inds, w, b)
    Q, X, TOTW = plan
    key = (
        MM_DT,
        OT_DT,
        N_WARM,
        WARM_N,
        STORE_GPSIMD_N,
        SPLIT_LAST,
        GROUP_ORDER,
        Q.tobytes(),
    )
    nc = _programs.get(key)
    if nc is None:
        nc = _build(Q, X, TOTW)
        _programs[key] = nc
    in_maps = [{"xt": xt[k], "wq": wqk[k], "bp": bpk[k]} for k in range(NCORES)]
    return nc, in_maps, tok_addr, order


def kernel(input, inds, w, b):
    x = np.ascontiguousarray(np.asarray(input, dtype=np.float32))
    inds = np.asarray(inds, dtype=np.int32)
    w = np.ascontiguousarray(np.asarray(w, dtype=np.float32))
    b = np.ascontiguousarray(np.asarray(b, dtype=np.float32))
    assert x.shape == (N_TOK, F) and inds.shape == (N_TOK,)
    assert w.shape == (E, F, O) and b.shape == (E, 1, O)

    try:
        nc, in_maps, tok_addr, order = _prepare(x, inds, w, b)
    except _CapacityOverflow:
        return (np.einsum("ni,nio->no", x, w[inds]) + b[inds, 0]).astype(np.float32)

    res = run_bass_kernel_spmd(nc, in_maps, list(range(NCORES)))
    return _unpack(res.results, tok_addr, order)


def last_program():
    """The most recently compiled Bass program (for profiling in test.py)."""
    return next(iter(_programs.values())) if _programs else None


# revision 57
# speedup vs baseline: 1.0320x; 1.0086x over previous
"""MoE routed expert matmul on 8 Trainium2 NeuronCores.

Problem: out[n] = input[n] @ w[inds[n]] + b[inds[n]]
  input [262144, 32] f32, inds [262144] i32 (1024 experts), w [1024, 32, 32], b [1024, 1, 32]

Strategy (K-stacked expert quads; host does routing/layout only — all FLOPs
on device):
  * Host sorts the 1024 experts by global token count (ascending) and chunks
    them into 32 quad-groups of 32 experts with near-equal counts.  Chunk q
    supplies one expert to each (core, band) pair: expert chunks[q][4k + r]
    goes to core k, quad q, band r (r in 0..3).  Every core runs the same
    program over its own 32 quads; quad q's column width Q[q] = max token
    count in the chunk (global max, so the SPMD shapes match), rounded up to
    2.  Count-matched chunks keep padding to ~2%.
  * Activation layout xt [128, TOTW] fp16: token t of (quad q, band r) sits
    at column X[q] + t, rows 32r..32r+32 (its 32 features).  Each column
    carries up to 4 tokens (one per band) — full 128-row density.
  * Weights upload as block-diagonal K=64 stacks (wq, ~0.5 MB): for each
    quad and half h, a [64, 64] tile holds experts (q, 2h) and (q, 2h+1) on
    the diagonal.  Two [K=64, M=64, N=Q] matmuls per quad (tile_position
    (0,0) / (64,64)) then compute all 4 bands' tokens — each activation
    column streams through the PE twice instead of 4x (vs per-expert 32x32
    tiles), and the off-diagonal zeros kill the cross-expert terms.  One
    small early group (K32_GROUPS) instead runs 4 per-expert K=32 matmuls,
    halving its weight bytes where the PE has load-phase slack.
  * Matmuls accumulate into pair-level PSUM tiles (2 quads x 512-col banks,
    4 in flight) so the bias latency stays out of the PSUM-recycle loop.
    Bias + fp16 down-convert runs split across engines per 4-quad group:
    ScalarE handles quads 0-1 as two exact-width activation ops, VectorE
    quads 2-3 in one tensor_tensor with a broadcast bias view.
  * Schedule: all xt tiles are SBUF-resident; loads issue upfront on the SP
    ring and run back-to-back, stores queue behind them (2 early groups on
    the GpSimd SWDGE ring, the rest on SP), so the DMA engines never idle
    and the compute tail hides inside the store backlog.  Group processing
    order pulls two small groups early so the drain-phase bias backlog
    never paces the final stores.  fp16 I/O halves DMA traffic vs f32;
    per-core HBM bytes ~4.8 MB -> ~13.5 us at 360 B/ns, sim 17.1 us.
  * Host scatters the sorted outputs back to original token order.

Layouts (core k, quad q, band r = 2h + s, expert e = chunks[q][4k + r]):
  xt [128, TOTW]  xt[32r+i, X[q] + t]        = x[token t of e, feat i]  (fp16)
  wq [128, 2048]  wq[64h+32s+i, 64q+32s+o]   = w[e, i, o], 0 off-diag   (fp16)
  bp [128, 32]    bp[32r+o, q]               = b[e, 0, o]               (fp16)
  ot [128, TOTW]  ot[32r+o, X[q] + t]        = out[token t of e, feat o](fp16)
"""

import numpy as np

import concourse.bass as bass
import concourse.mybir as mybir
import concourse.tile as tile
from concourse import bacc
from concourse.bass_utils import run_bass_kernel_spmd

N_TOK = 262144
E = 1024
F = 32
O = 32
NCORES = 8
NQUAD = 32  # quads per core; 4 experts each = 128 experts/core
GQ = 4  # quads per load/store group
NG = NQUAD // GQ
F32 = mybir.dt.float32
MM_DT = mybir.dt.float16
OT_DT = mybir.dt.float16

N_WARM = 6  # PE ramp warm-up matmuls
WARM_N = 160  # free-dim length of each warm-up matmul
STORE_GPSIMD_N = 2  # leading store groups on the Pool SWDGE ring (rest: SP)
SPLIT_LAST = True  # split the last group's load at its final quad
# group processing order (indices into size-descending groups): two small
# groups early so the engine bias backlog never paces the store drain
GROUP_ORDER = (0, 5, 7, 1, 2, 3, 4, 6)
# processed-group indices whose quads run as 4 per-expert K=32 matmuls
# (half the weight-upload bytes, double the PE rows) — small early groups
# where the PE has load-phase slack
K32_GROUPS = (1,)

_programs: dict[tuple, "bacc.Bacc"] = {}


class _CapacityOverflow(Exception):
    """A single expert got >512 tokens (~16 sigma out for uniform routing at
    256 tokens/expert).  Handled by a host fallback so kernel() still
    returns a correct result."""


def _plan(counts):
    """Chunk experts into count-matched quads; per-quad widths and offsets."""
    order_e = np.argsort(counts, kind="stable")  # ascending counts
    # chunk q holds 32 count-matched experts; descending so the pipeline
    # tail (last-stored groups) drains on the smallest transfers
    chunks = order_e.reshape(NQUAD, 32)[::-1]
    # optional group-level processing permutation (load/compute/store order)
    chunks = chunks.reshape(NG, GQ, 32)[list(GROUP_ORDER)].reshape(NQUAD, 32)
    Q = np.maximum(16, ((counts[chunks[:, -1]] + 7) // 8) * 8)  # [NQUAD]
    # quads 2-3 of each group share one batched DVE bias op, which needs a
    # uniform column stride — pad that pair to its max (sorted chunks make
    # this ~0.5%); quads 0-1 get per-quad ACT ops and stay exact
    Q4 = Q.reshape(NG, GQ).copy()
    Q4[:, 2:4] = Q4[:, 2:4].max(axis=1, keepdims=True)
    Q = Q4.reshape(-1)
    if Q.max() > 512:
        raise _CapacityOverflow(int(counts.max()))
    X = np.zeros(NQUAD + 1, dtype=np.int64)
    np.cumsum(Q, out=X[1:])
    TOTW = int(X[-1])
    j = np.arange(32)
    e_quad = np.empty(E, dtype=np.int64)
    e_core = np.empty(E, dtype=np.int64)
    e_band = np.empty(E, dtype=np.int64)
    e_quad[chunks] = np.arange(NQUAD)[:, None]
    e_core[chunks] = (j // 4)[None, :]
    e_band[chunks] = (j % 4)[None, :]
    k32 = np.zeros(NQUAD, dtype=bool)
    for g in K32_GROUPS:
        k32[GQ * g : GQ * (g + 1)] = True
    wqw = np.where(k32, 32, 64)
    wqX = np.zeros(NQUAD + 1, dtype=np.int64)
    np.cumsum(wqw, out=wqX[1:])
    return Q.astype(np.int64), X, TOTW, e_quad, e_core, e_band, k32, wqX


def _build(Q, X, TOTW, k32, wqX) -> "bacc.Bacc":
    WQW = int(wqX[-1])
    nc = bacc.Bacc("TRN2", target_bir_lowering=False, debug=False, num_devices=NCORES)
    xt = nc.declare_dram_parameter("xt", [128, TOTW], MM_DT, isOutput=False)
    wq = nc.declare_dram_parameter("wq", [128, WQW], MM_DT, isOutput=False)
    bp = nc.declare_dram_parameter("bp", [128, NQUAD], MM_DT, isOutput=False)
    ot = nc.declare_dram_parameter("ot", [128, TOTW], OT_DT, isOutput=True)

    with tile.TileContext(nc) as tc:
        with (
            tc.tile_pool(name="const", bufs=1) as c_pool,
            tc.tile_pool(name="xt", bufs=NG) as xt_pool,
            tc.tile_pool(name="out", bufs=NG) as out_pool,
            tc.tile_pool(name="psm", bufs=4, space="PSUM") as psm_pool,
        ):
            wq_t = c_pool.tile([128, WQW], MM_DT)
            bp_t = c_pool.tile([128, NQUAD], MM_DT)
            warm_t = c_pool.tile([128, WARM_N], MM_DT)

            # loads: gpsimd (SWDGE, 25ns SEQ issue) carries wq in chunks —
            # groups 0-1 first so compute starts early — plus the bias;
            # sync (SP HWDGE) carries all xt
            wq2g = int(wqX[2 * GQ])
            nc.gpsimd.dma_start(out=wq_t[:, :wq2g], in_=wq[:, :wq2g])
            nc.gpsimd.dma_start(out=bp_t[:], in_=bp[:])
            nc.gpsimd.dma_start(out=wq_t[:, wq2g:], in_=wq[:, wq2g:])

            # PE ramp warm-up on a memset scratch tile (PSUM never read);
            # the dummy activation pulls ACT's 1.3us LoadActFuncSet into the
            # load phase instead of stalling the first real bias op
            nc.vector.memset(warm_t[:], 0.0)
            nc.scalar.activation(
                warm_t[0:1, 0:1],
                warm_t[0:1, 0:1],
                mybir.ActivationFunctionType.Identity,
                bias=warm_t[0:1, 1:2],
                scale=1.0,
            )
            warm_ps = psm_pool.tile(
                [128, WARM_N], F32, space="PSUM", name="warm_ps", tag="psm"
            )
            for _ in range(N_WARM):
                nc.tensor.matmul(
                    out=warm_ps[0:32, :],
                    lhsT=warm_t[0:32, 0:32],
                    rhs=warm_t[0:32, :],
                    start=True,
                    stop=True,
                    tile_position=(0, 0),
                )

            xt_tiles = {}
            o_tiles = {}

            def load_group(g, cuts=()):
                a, bnd = int(X[GQ * g]), int(X[GQ * (g + 1)])
                t = xt_pool.tile([128, bnd - a], MM_DT, name="xt_t", tag="xt_t")
                for c0, c1 in zip((a, *cuts), (*cuts, bnd)):
                    nc.sync.dma_start(
                        out=t[:, c0 - a : c1 - a], in_=xt[:, c0:c1]
                    )
                xt_tiles[g] = t

            # all xt tiles are resident; loads issue upfront and run
            # back-to-back so stores queue behind them and the compute tail
            # hides inside the store backlog
            # group 0 split after its second quad: the first piece's
            # transfer exactly covers the Pool ring's wq prep latency so the
            # DMA engines never idle between the two
            load_group(0, cuts=(int(X[2]),))
            for g in range(1, NG - 1):
                load_group(g)
            # keep the trailing piece >= 256 cols (512B rows) so it doesn't
            # pay the sub-512B descriptor latency penalty
            last_cut = min(int(X[NQUAD - 1]), int(X[NQUAD]) - 256)
            split_ok = SPLIT_LAST and last_cut > int(X[GQ * (NG - 1)])
            load_group(NG - 1, cuts=(last_cut,) if split_ok else ())

            for g in range(NG):
                a, bnd = int(X[GQ * g]), int(X[GQ * (g + 1)])
                o_t = out_pool.tile([128, bnd - a], OT_DT, name="o_t", tag="o_t")
                # pair-level PSUM tiles (2 banks each, 4 in flight) keep the
                # bias latency out of the PSUM-recycle critical loop
                for pi in range(2):
                    psm = psm_pool.tile(
                        [128, 2 * 512], F32, space="PSUM", name="psm", tag="psm"
                    )
                    for si in range(2):
                        q = GQ * g + 2 * pi + si
                        Qq = int(Q[q])
                        off = int(X[q] - a)
                        wx = int(wqX[q])
                        if k32[q]:
                            for r in range(4):
                                nc.tensor.matmul(
                                    out=psm[
                                        32 * r : 32 * r + 32,
                                        512 * si : 512 * si + Qq,
                                    ],
                                    lhsT=wq_t[32 * r : 32 * r + 32, wx : wx + 32],
                                    rhs=xt_tiles[g][
                                        32 * r : 32 * r + 32, off : off + Qq
                                    ],
                                    start=True,
                                    stop=True,
                                    tile_position=(32 * r, 32 * r),
                                )
                        else:
                            for h in range(2):
                                nc.tensor.matmul(
                                    out=psm[
                                        64 * h : 64 * h + 64,
                                        512 * si : 512 * si + Qq,
                                    ],
                                    lhsT=wq_t[64 * h : 64 * h + 64, wx : wx + 64],
                                    rhs=xt_tiles[g][
                                        64 * h : 64 * h + 64, off : off + Qq
                                    ],
                                    start=True,
                                    stop=True,
                                    tile_position=(64 * h, 64 * h),
                                )
                    # bias + fp16 down-convert: ACT takes the first pair as
                    # two exact-width activation ops (it is the serial/slower
                    # engine, so give it the earlier-ready pair); DVE takes
                    # the second pair in one tensor_tensor (broadcast bias,
                    # pair-uniform width) — parallel engines, different banks
                    if pi == 0:
                        for si in range(2):
                            q = GQ * g + si
                            Qq = int(Q[q])
                            off = int(X[q] - a)
                            nc.scalar.activation(
                                o_t[:, off : off + Qq],
                                psm[:, 512 * si : 512 * si + Qq],
                                mybir.ActivationFunctionType.Identity,
                                bias=bp_t[:, q : q + 1],
                                scale=1.0,
                            )
                    else:
                        Qp = int(Q[GQ * g + 2])
                        off = int(X[GQ * g + 2] - a)
                        psm_view = psm[:, :].rearrange("p (c t) -> p c t", c=2)[
                            :, :, :Qp
                        ]
                        bias_view = bp_t[
                            :, GQ * g + 2 : GQ * g + 4, None
                        ].to_broadcast([128, 2, Qp])
                        out_view = o_t[:, off : off + 2 * Qp].rearrange(
                            "p (c t) -> p c t", c=2
                        )
                        nc.vector.tensor_tensor(
                            out=out_view,
                            in0=psm_view,
                            in1=bias_view,
                            op=mybir.AluOpType.add,
                        )
                # stores: leading groups on the Pool SWDGE ring (25ns SEQ
                # issue, never blocks compute); the rest on the SP ring where
                # they queue behind the loads (loads keep DMA priority) and
                # get the lower-latency HWDGE issue path for the tail
                ring = nc.gpsimd if g < STORE_GPSIMD_N else nc.sync
                ring.dma_start(out=ot[:, a:bnd], in_=o_t[:])

    nc.compile()
    return nc


def _pack(x, inds, w, b):
    """Host-side routing: sort tokens by expert, build per-core device arrays."""
    counts = np.bincount(inds, minlength=E)
    Q, X, TOTW, e_quad, e_core, e_band, k32, wqX = _plan(counts)

    order = np.argsort(inds, kind="stable")
    sorted_inds = inds[order]
    starts = np.zeros(E, dtype=np.int64)
    np.cumsum(counts[:-1], out=starts[1:])
    slot = np.arange(N_TOK, dtype=np.int64) - starts[sorted_inds]

    k_tok = e_core[sorted_inds]
    r_tok = e_band[sorted_inds]
    col_tok = X[e_quad[sorted_inds]] + slot

    mdt = mybir.dt.np(MM_DT)
    xt_all = np.zeros((NCORES, 4, F, TOTW), dtype=mdt)
    xt_all[k_tok, r_tok, :, col_tok] = x[order].astype(mdt)
    xt = xt_all.reshape(NCORES, 128, TOTW)

    # per-quad weight blocks: K=64 quads get two [64, 64] diagonal tiles
    # (partition halves), K=32 quads one [32, 32] block per band
    WQW = int(wqX[-1])
    wqk = np.zeros((NCORES, 128, WQW), dtype=mdt)
    wf = w.astype(mdt)
    for e in range(E):
        k, q, r = int(e_core[e]), int(e_quad[e]), int(e_band[e])
        wx = int(wqX[q])
        if k32[q]:
            wqk[k, 32 * r : 32 * r + 32, wx : wx + 32] = wf[e]
        else:
            h, sdx = r // 2, r % 2
            wqk[
                k,
                64 * h + 32 * sdx : 64 * h + 32 * sdx + 32,
                wx + 32 * sdx : wx + 32 * sdx + 32,
            ] = wf[e]

    bpn = np.zeros((NCORES, 4, O, NQUAD), dtype=mdt)
    bpn[e_core, e_band, :, e_quad] = b[:, 0, :].astype(mdt)
    bpk = bpn.reshape(NCORES, 128, NQUAD)

    plan = (Q, X, TOTW, k32, wqX)
    return plan, order, (k_tok, r_tok, col_tok), xt, wqk, bpk


def _unpack(results, tok_addr, order):
    k_tok, r_tok, col_tok = tok_addr
    ot = np.stack([results[k]["ot"] for k in range(NCORES)])  # [k, 128, TOTW]
    ot4 = ot.reshape(NCORES, 4, O, -1)  # [k, r, o, col]
    out = np.empty((N_TOK, O), dtype=np.float32)
    out[order] = ot4[k_tok, r_tok, :, col_tok]
    return out


def _prepare(x, inds, w, b):
    """Pack inputs and return (nc, in_maps, tok_addr, order)."""
    plan, order, tok_addr, xt, wqk, bpk = _pack(x, inds, w, b)
    Q, X, TOTW, k32, wqX = plan
    key = (
        MM_DT,
        OT_DT,
        N_WARM,
        WARM_N,
        STORE_GPSIMD_N,
        SPLIT_LAST,
        GROUP_ORDER,
        K32_GROUPS,
        Q.tobytes(),
    )
    nc = _programs.get(key)
    if nc is None:
        nc = _build(Q, X, TOTW, k32, wqX)
        _programs[key] = nc
    in_maps = [{"xt": xt[k], "wq": wqk[k], "bp": bpk[k]} for k in range(NCORES)]
    return nc, in_maps, tok_addr, order


def kernel(input, inds, w, b):
    x = np.ascontiguousarray(np.asarray(input, dtype=np.float32))
    inds = np.asarray(inds, dtype=np.int32)
    w = np.ascontiguousarray(np.asarray(w, dtype=np.float32))
    b = np.ascontiguousarray(np.asarray(b, dtype=np.float32))
    assert x.shape == (N_TOK, F) and inds.shape == (N_TOK,)
    assert w.shape == (E, F, O) and b.shape == (E, 1, O)

    try:
        nc, in_maps, tok_addr, order = _prepare(x, inds, w, b)
    except _CapacityOverflow:
        return (np.einsum("ni,nio->no", x, w[inds]) + b[inds, 0]).astype(np.float32)

    res = run_bass_kernel_spmd(nc, in_maps, list(range(NCORES)))
    return _unpack(res.results, tok_addr, order)


def last_program():
    """The most recently compiled Bass program (for profiling in test.py)."""
    return next(iter(_programs.values())) if _programs else None
